# revision 124
# baseline (speedup 1.0000x reference)
"""DNeRF renderer on 8 Trainium2 cores (Bass/Tile).

Data-parallel over rays (1024 rays/core, 8 ray-tiles of 128 rays).
Per-ray sort/searchsorted machinery is done with closed-form merge ranks
(the coarse z grid is uniform per ray) + GPSIMD local_scatter
(per-partition scatter of f32 values as int16 pairs) + tensor_tensor_scan
(per-partition prefix scans).  The density/color MLPs run on the PE in a
transposed layout (HBM bridge), and are *recomputed* at the merged-sorted
z positions instead of permuting payloads (the image is an order-free
weighted sum over samples).
"""

import numpy as np
from contextlib import ExitStack

import concourse.bass as bass
import concourse.bacc as bacc
import concourse.mybir as mybir
import concourse.tile as tile
from concourse.bass_utils import run_bass_kernel_spmd
from concourse import library_config

dt = mybir.dt
Alu = mybir.AluOpType
Act = mybir.ActivationFunctionType
AxX = mybir.AxisListType.X


def f32r(ap):
    """Relaxed-fp32 matmul mode: 1 PE cycle/row (vs 4 for strict fp32)."""
    return ap.bitcast(dt.float32r)

NCORES = 8
NRAYS = 8192
R = NRAYS // NCORES      # rays per core
P = 128                  # rays per tile (partitions)
T = R // P               # ray-tiles per core
S = 64                   # num_steps
U = 64                   # upsample_steps
M = S + U                # merged samples
MIN_NEAR = 0.05
DS = 1.0                 # DENSITY_SCALE
M24 = 16777216.0         # 2^24

_BUILT = None


def _build():
    nc = bacc.Bacc("TRN2", target_bir_lowering=False, debug=False,
                   num_devices=NCORES)

    def din(name, shape):
        return nc.dram_tensor(name, shape, dt.float32, kind="ExternalInput").ap()

    rays_o = din("rays_o_k", [P, T, 3])
    rays_d = din("rays_d_k", [P, T, 3])
    dT8_in = din("dT8_k", [T, 8, 64])
    dlhs8 = din("dlhs8", [8, 128])
    v128 = din("v128", [P, S])
    iota_r = din("iota_r", [P, M])
    iotap1_r = din("iotap1_r", [P, M])
    iotaev62 = din("iotaev62", [P, 62])
    zero128 = din("zero128", [P, M])
    cc = din("cc", [P, 12])
    ones_row = din("ones_row", [1, P * M])
    lhsT7 = din("lhsT7", [7, 128])
    w0oct = din("w0oct", [128, 4, 8])
    wgcpair = din("wgcpair", [128, 128])
    wc2oct = din("wc2oct", [128, 4, 24])
    bgrep = din("bgrep", [P, 3])
    bc2rep = din("bc2rep", [P, 3])
    scl_in = din("scl", [P, 4])

    img_out = nc.dram_tensor("img_k", [P, T, 3], dt.float32,
                             kind="ExternalOutput").ap()

    fh = P * M // 2   # 8192 final pair-cols per tile
    half = P * S // 2  # 4096 coarse pair-cols per tile

    with tile.TileContext(nc) as tc, ExitStack() as ctx:

        cpool = ctx.enter_context(tc.tile_pool(name="consts", bufs=1))
        spool = ctx.enter_context(tc.tile_pool(name="setup", bufs=1))
        wpool = ctx.enter_context(tc.tile_pool(name="work", bufs=2))
        gpool = ctx.enter_context(tc.tile_pool(name="gather", bufs=1))
        bpool = ctx.enter_context(tc.tile_pool(name="big", bufs=1))
        ppool = ctx.enter_context(tc.tile_pool(name="psum", bufs=3, space="PSUM"))
        pspool = ctx.enter_context(tc.tile_pool(name="psum_s", bufs=2, space="PSUM"))
        dpool = ctx.enter_context(tc.tile_pool(name="dram", bufs=2, space="DRAM"))

        def cload(ap_in, shape, tag, dtype=dt.float32, mm=False):
            t_ = cpool.tile(shape, dtype, tag=tag, name=tag)
            if mm:
                nc.sync.dma_start(f32r(t_[:]), f32r(ap_in))
            else:
                nc.sync.dma_start(t_[:], ap_in)
            return t_

        # ray data + cc first: stage A and tile 0 depend on these
        ro_s = cload(rays_o, [P, T, 3], tag='c_rays_o')
        rd_s = cload(rays_d, [P, T, 3], tag='c_rays_d')
        cc_s = cload(cc, [P, 12], tag='c_cc')
        v128_s = cload(v128, [P, S], tag='c_v128')
        lhsT7_s = cload(lhsT7, [7, 128], tag='c_lhsT7', mm=True)
        w0p_s = cload(w0oct, [128, 4, 8], tag='c_w0oct', mm=True)
        scl_s = cload(scl_in, [P, 4], tag='c_scl_in')
        zero_s = cload(zero128, [P, M], tag='c_zero128')
        iop1_s = cload(iotap1_r, [P, M], tag='c_iotap1_r')
        iev62_s = cload(iotaev62, [P, 62], tag='c_iotaev62')
        wgc_s = cload(wgcpair, [128, 128], tag='c_wgcpair', mm=True)
        wc2_s = cload(wc2oct, [128, 4, 24], tag='c_wc2oct', mm=True)
        dlhs8_s = cload(dlhs8, [8, 128], tag='c_dlhs8', mm=True)
        bg_s = cload(bgrep, [P, 3], tag='c_bgrep')
        bc2_s = cload(bc2rep, [P, 3], tag='c_bc2rep')

        def bc(col, n):
            return col.broadcast_to((P, n))

        ones_c = cc_s[:, 0:1]
        neg1_c = cc_s[:, 1:2]
        eps_c = cc_s[:, 2:3]
        e15_c = cc_s[:, 3:4]
        e5_c = cc_s[:, 4:5]
        nhalf_c = cc_s[:, 5:6]
        nm24_c = cc_s[:, 6:7]
        n2_c = cc_s[:, 7:8]
        m24_c = cc_s[:, 8:9]
        bd2_0c = scl_s[:, 0:1]

        # moving-tensor buffers for the MLP matmuls; row layout
        # (xA xB yA yB zA zB 1).  ones rows written once, reused by all tiles.
        # separate coarse/fine buffers so tile t+1's coarse pass can overlap
        # tile t's fine pass.
        rhs7c = bpool.tile([7, half], dt.float32, tag="rhs7co", name="rhs7co")
        nc.sync.dma_start(f32r(rhs7c[6:7, :]), f32r(ones_row[:, 0:half]))
        rhs7 = bpool.tile([7, fh], dt.float32, tag="rhs7", name="rhs7")
        nc.sync.dma_start(f32r(rhs7[6:7, :]), f32r(ones_row[:, 0:fh]))

        # ================= STAGE A: ray setup (all tiles) ================
        n24 = T * 3

        def st(shape, tag, dtype=dt.float32):
            return spool.tile(shape, dtype, tag=tag, name=tag)

        negd = st([P, T, 3], 's_negd')
        nc.vector.tensor_scalar(negd[:], rd_s[:], -1.0, None, Alu.mult)
        absd = st([P, T, 3], 's_absd')
        nc.vector.tensor_tensor(absd[:], rd_s[:], negd[:], Alu.max)
        dmask = st([P, T, 3], 's_dmask', dt.uint8)
        nc.vector.tensor_scalar(dmask[:], absd[:], 1e-9, None, Alu.is_lt)
        dsafe = st([P, T, 3], 's_dsafe')
        nc.vector.select(dsafe[:].rearrange("p t c -> p (t c)"),
                         dmask[:].rearrange("p t c -> p (t c)"),
                         bc(eps_c, n24),
                         rd_s[:].rearrange("p t c -> p (t c)"))
        invd = st([P, T, 3], 's_invd')
        nc.vector.reciprocal(invd[:], dsafe[:])
        a1 = st([P, T, 3], 's_a1')
        nc.vector.scalar_tensor_tensor(a1[:], ro_s[:], 1.0, invd[:],
                                       Alu.add, Alu.mult)
        b1 = st([P, T, 3], 's_b1')
        nc.vector.scalar_tensor_tensor(b1[:], ro_s[:], -1.0, invd[:],
                                       Alu.add, Alu.mult)
        mx = st([P, T, 3], 's_mx')
        nc.vector.tensor_tensor(mx[:], a1[:], b1[:], Alu.max)
        mn = st([P, T, 3], 's_mn')
        nc.vector.tensor_tensor(mn[:], a1[:], b1[:], Alu.min)
        tmin = st([P, T], 's_tmin')
        nc.vector.tensor_reduce(tmin[:], mx[:], AxX, Alu.min)
        tmax = st([P, T], 's_tmax')
        nc.vector.tensor_reduce(tmax[:], mn[:], AxX, Alu.max)
        near = st([P, T], 's_near')
        nc.vector.tensor_scalar(near[:], tmin[:], -1.0, MIN_NEAR,
                                Alu.mult, Alu.max)
        tmaxt = st([P, T], 's_tmaxt')
        nc.vector.tensor_scalar(tmaxt[:], tmax[:], -1.0, None, Alu.mult)
        fmask = st([P, T], 's_fmask', dt.uint8)
        nc.vector.tensor_tensor(fmask[:], tmaxt[:], near[:], Alu.is_lt)
        nearp = st([P, T], 's_nearp')
        nc.vector.tensor_scalar(nearp[:], near[:], 1e-2, None, Alu.add)
        far = st([P, T], 's_far')
        nc.vector.select(far[:], fmask[:], nearp[:], tmaxt[:])
        rng = st([P, T], 's_rng')
        nc.vector.tensor_tensor(rng[:], far[:], near[:], Alu.subtract)
        dzv = st([P, T], 's_dzv')
        nc.vector.tensor_scalar(dzv[:], rng[:], 1.0 / 63.0, None, Alu.mult)
        sdv = st([P, T], 's_sdv')
        nc.vector.tensor_scalar(sdv[:], rng[:], 1.0 / 64.0, None, Alu.mult)
        invdz = st([P, T], 's_invdz')
        nc.vector.reciprocal(invdz[:], dzv[:])
        inv2dz = st([P, T], 's_inv2dz')
        nc.vector.tensor_scalar(inv2dz[:], invdz[:], 2.0, None, Alu.mult)
        mid0 = st([P, T], 's_mid0')
        nc.vector.scalar_tensor_tensor(mid0[:], dzv[:], 0.5, near[:],
                                       Alu.mult, Alu.add)
        aoff = st([P, T], 's_aoff')
        nc.vector.tensor_tensor(aoff[:], near[:], dzv[:], Alu.subtract)

        # ================= per ray-tile ==================================
        # software pipelined: phase1 = coarse MLP + sampling machinery,
        # phase2 = fine MLP + composite.  phase1(t+1) is issued before
        # phase2(t) so the in-order engine queues overlap tiles.
        def wt(shape, tag, dtype=dt.float32):
            return wpool.tile(shape, dtype, tag=tag, name=tag)

        state = {}

        def phase1a(t):
            near_c = near[:, t:t + 1]
            rng_c = rng[:, t:t + 1]
            dz_c = dzv[:, t:t + 1]
            sd_c = sdv[:, t:t + 1]
            i2dz_c = inv2dz[:, t:t + 1]
            mid0_c = mid0[:, t:t + 1]
            aoff_c = aoff[:, t:t + 1]

            # ---- coarse z, deltas, xyz ----
            zc = wt([P, S], "zc")
            nc.vector.scalar_tensor_tensor(zc[:], v128_s[:], rng_c,
                                           bc(near_c, S), Alu.mult, Alu.add)
            deltas = wt([P, S], "deltas")
            nc.vector.tensor_tensor(deltas[:, 0:S - 1], zc[:, 1:S],
                                    zc[:, 0:S - 1], Alu.subtract)
            nc.scalar.copy(deltas[:, S - 1:S], sd_c)

            xyzc = wt([P, 3, S], "xyzc")
            for c in range(3):
                nc.vector.scalar_tensor_tensor(
                    xyzc[:, c, :], zc[:], rd_s[:, t, c:c + 1],
                    bc(ro_s[:, t, c:c + 1], S), Alu.mult, Alu.add)
            nc.vector.scalar_tensor_tensor(
                xyzc[:].rearrange("p c s -> p (c s)"),
                xyzc[:].rearrange("p c s -> p (c s)"),
                1.0, bc(neg1_c, 3 * S), Alu.min, Alu.max)

            # ---- direct SBUF->SBUF transpose into the matmul layout ----
            # (Act HWDGE queue: keeps the critical xyz DMAs out of the SP
            # queue where sigma/rgb DMAs wait on late producers)
            for c in range(3):
                nc.scalar.dma_start(f32r(rhs7c[2 * c:2 * c + 2, :]),
                                    f32r(xyzc[:, c, :]))

            rh1 = bpool.tile([128, half], dt.float32, tag="rh1co", name="rh1co")
            for g in range(4):
                pA = ppool.tile([128, 1024], dt.float32, tag="mm", name="pmm")
                for ch in range(2):
                    nc.tensor.matmul(pA[:, 512 * ch:512 * (ch + 1)],
                                     f32r(lhsT7_s[:]),
                                     f32r(rhs7c[:, g * 1024 + 512 * ch:
                                                g * 1024 + 512 * (ch + 1)]),
                                     start=True, stop=True)
                nc.scalar.activation(f32r(rh1[:, g * 1024:(g + 1) * 1024]),
                                     pA[:], Act.Relu)

            h20 = wt([P, S], "h20")
            sgco = gpool.tile([8, 1024], dt.float32, tag="sgco", name="sgco")
            for g in range(2):
                pS = pspool.tile([8, 512], dt.float32, tag="ps", name="pps")
                for ch in range(4):
                    nc.tensor.matmul(pS[:],
                                     f32r(w0p_s[:, ch, :]),
                                     f32r(rh1[:, (2 * ch + g) * 512:
                                              (2 * ch + g + 1) * 512]),
                                     start=(ch == 0), stop=(ch == 3))
                nc.scalar.copy(sgco[:, 512 * g:512 * (g + 1)], pS[:])
            for h in range(2):
                nc.scalar.dma_start(h20[64 * h:64 * h + 64, :],
                                    sgco[4 * h:4 * h + 4, :])
            state[t] = (h20, deltas)

        def phase1b(t):
            h20, deltas = state.pop(t)
            near_c = near[:, t:t + 1]
            rng_c = rng[:, t:t + 1]
            dz_c = dzv[:, t:t + 1]
            sd_c = sdv[:, t:t + 1]
            i2dz_c = inv2dz[:, t:t + 1]
            mid0_c = mid0[:, t:t + 1]
            aoff_c = aoff[:, t:t + 1]

            # ---- coarse composite weights (machinery on gpsimd/Pool) ----
            sig = wt([P, S], "sig")
            nc.scalar.activation(sig[:], h20[:], Act.Exp, bias=bd2_0c)
            dsg = wt([P, S], "dsg")
            nc.gpsimd.tensor_tensor(dsg[:], deltas[:], sig[:], Alu.mult)
            em = wt([P, S], "em")
            nc.scalar.activation(em[:], dsg[:], Act.Exp, scale=-DS)
            sbuf_t = wt([P, S], "sbuft")
            nc.gpsimd.memset(sbuf_t[:, 0:1], 1.0)
            nc.scalar.activation(sbuf_t[:, 1:S], em[:, 0:S - 1],
                                 Act.Identity, bias=e15_c)
            Tc = wt([P, S], "Tcz")
            nc.vector.tensor_tensor_scan(Tc[:], sbuf_t[:], zero_s[:, 0:S],
                                         1.0, Alu.mult, Alu.add)
            alpha = wt([P, S], "alpha")
            nc.gpsimd.tensor_scalar(alpha[:], em[:], -1.0, 1.0,
                                    Alu.mult, Alu.add)
            wts = wt([P, S], "wts")
            nc.gpsimd.tensor_tensor(wts[:], alpha[:], Tc[:], Alu.mult)

            # ---- pdf/cdf over weights[:,1:63] ----
            wp = wt([P, 62], "wp")
            nc.scalar.activation(wp[:], wts[:, 1:63], Act.Identity, bias=e5_c)
            ssum = wt([P, 1], "ssum")
            nc.vector.tensor_reduce(ssum[:], wp[:], AxX, Alu.add)
            pinv = wt([P, 1], "pinv")
            nc.vector.reciprocal(pinv[:], ssum[:])
            pdf = wt([P, 62], "pdf")
            nc.vector.tensor_scalar(pdf[:], wp[:], pinv[:], None, Alu.mult)
            cdf = wt([P, 62], "cdf")
            nc.vector.tensor_tensor_scan(cdf[:], pdf[:], zero_s[:, 0:62],
                                         0.0, Alu.add, Alu.add)

            # ---- scatter cdf onto 128-slot (cdf ∪ u) timeline ----
            r2 = wt([P, 62], "r2")
            nc.gpsimd.tensor_scalar(r2[:], cdf[:], 128.0, M24,
                                    Alu.mult, Alu.add)
            nc.gpsimd.tensor_scalar(r2[:], r2[:], -M24, None, Alu.add)
            idx2f = wt([P, 124], "idx2f")
            ev = idx2f[:].rearrange("p (a b) -> p a b", b=2)[:, :, 0:1] \
                .rearrange("p a b -> p (a b)")
            od = idx2f[:].rearrange("p (a b) -> p a b", b=2)[:, :, 1:2] \
                .rearrange("p a b -> p (a b)")
            nc.gpsimd.tensor_tensor(ev, r2[:], iev62_s[:], Alu.add)
            nc.gpsimd.tensor_scalar(od, ev, 1.0, None, Alu.add)
            idx2i = wt([P, 124], "idx2i", dt.int16)
            nc.gpsimd.tensor_copy(idx2i[:], idx2f[:])
            tlc2 = wt([P, 256], "tlc2", dt.int16)
            nc.gpsimd.local_scatter(tlc2[:], cdf[:].bitcast(dt.int16),
                                    idx2i[:], channels=P, num_elems=256,
                                    num_idxs=124)
            tlc = tlc2[:].bitcast(dt.float32)

            # ---- fills and counts on the timeline ----
            notC = wt([P, M], "notC")
            nc.gpsimd.tensor_scalar(notC[:], tlc, 0.0, None, Alu.is_equal)
            kp1 = wt([P, M], "kp1")
            nc.vector.tensor_tensor_scan(kp1[:], notC[:], zero_s[:],
                                         0.0, Alu.add, Alu.add)
            uu = wt([P, M], "uu")
            nc.gpsimd.tensor_scalar(uu[:], kp1[:], 1.0 / 64.0, -1.0 / 128.0,
                                    Alu.mult, Alu.add)
            cntC = wt([P, M], "cntC")
            nc.gpsimd.tensor_tensor(cntC[:], iop1_s[:], kp1[:], Alu.subtract)
            ffwd = wt([P, M], "ffwd")
            nc.vector.tensor_tensor_scan(ffwd[:], notC[:], tlc, 0.0,
                                         Alu.mult, Alu.add)
            rnotC = wt([P, M], "rnotC")
            nc.gpsimd.tensor_copy(rnotC[:], notC[:, ::-1])
            rtlc = wt([P, M], "rtlc")
            nc.gpsimd.tensor_copy(rtlc[:], tlc[:, ::-1])
            rbwd = wt([P, M], "rbwd")
            nc.vector.tensor_tensor_scan(rbwd[:], rnotC[:], rtlc[:], 0.0,
                                         Alu.mult, Alu.add)
            bwd = rbwd[:, ::-1]

            # ---- inverse-CDF lerp at u slots ----
            den = wt([P, M], "den")
            nc.gpsimd.tensor_tensor(den[:], bwd, ffwd[:], Alu.subtract)
            mkd = wt([P, M], "mkd", dt.uint8)
            nc.gpsimd.tensor_scalar(mkd[:], den[:], 1e-5, None, Alu.is_lt)
            nc.vector.select(den[:], mkd[:], bc(ones_c, M), den[:])
            rden = wt([P, M], "rden")
            nc.vector.reciprocal(rden[:], den[:])
            tt = wt([P, M], "tt")
            nc.gpsimd.tensor_tensor(tt[:], uu[:], ffwd[:], Alu.subtract)
            nc.gpsimd.tensor_tensor(tt[:], tt[:], rden[:], Alu.mult)
            bg0 = wt([P, M], "bg0")
            nc.vector.scalar_tensor_tensor(bg0[:], cntC[:], dz_c,
                                           bc(mid0_c, M), Alu.mult, Alu.add)
            nz = wt([P, M], "nz")
            nc.vector.scalar_tensor_tensor(nz[:], tt[:], dz_c, bg0[:],
                                           Alu.mult, Alu.add)

            # ---- merge ranks into final (coarse ∪ fine) timeline ----
            q2 = wt([P, M], "q2")
            nc.vector.scalar_tensor_tensor(q2[:], nz[:], near_c,
                                           bc(i2dz_c, M), Alu.subtract,
                                           Alu.mult)
            nc.gpsimd.tensor_scalar(q2[:], q2[:], 1.0, M24, Alu.add, Alu.add)
            nc.gpsimd.tensor_scalar(q2[:], q2[:], M24, 0.0, Alu.subtract,
                                    Alu.max)
            nc.gpsimd.tensor_scalar(q2[:], q2[:], 126.0, None, Alu.min)
            tk2 = wt([P, M], "tk2")
            nc.gpsimd.tensor_scalar(tk2[:], kp1[:], 2.0, -2.0,
                                    Alu.mult, Alu.add)
            mk2 = wt([P, M], "mk2")
            nc.gpsimd.tensor_scalar(mk2[:], kp1[:], 64.5, None, Alu.is_gt)
            minv = wt([P, M], "minv")
            nc.gpsimd.tensor_tensor(minv[:], mk2[:], notC[:], Alu.subtract)
            nc.gpsimd.tensor_scalar(minv[:], minv[:], 1.0, None, Alu.add)
            m2 = wt([P, M], "m2")
            nc.vector.scalar_tensor_tensor(m2[:], minv[:], -4000.0, q2[:],
                                           Alu.mult, Alu.add)
            ms = wt([P, M], "ms")
            nc.vector.tensor_tensor_scan(ms[:], m2[:], m2[:], -1e30,
                                         Alu.max, Alu.max)
            rk = wt([P, M], "rk")
            nc.gpsimd.tensor_tensor(rk[:], tk2[:], ms[:], Alu.add)
            nc.gpsimd.tensor_scalar(rk[:], rk[:], 254.0, None, Alu.min)
            nc.vector.scalar_tensor_tensor(rk[:], minv[:], -4000.0, rk[:],
                                           Alu.mult, Alu.add)
            fidx2f = wt([P, 256], "fidx2f")
            fev = fidx2f[:].rearrange("p (a b) -> p a b", b=2)[:, :, 0:1] \
                .rearrange("p a b -> p (a b)")
            fod = fidx2f[:].rearrange("p (a b) -> p a b", b=2)[:, :, 1:2] \
                .rearrange("p a b -> p (a b)")
            nc.gpsimd.tensor_copy(fev, rk[:])
            nc.gpsimd.tensor_scalar(fod, rk[:], 1.0, None, Alu.add)
            fidx2i = wt([P, 256], "fidx2i", dt.int16)
            nc.gpsimd.tensor_copy(fidx2i[:], fidx2f[:])
            zf2 = wt([P, 256], "zf2", dt.int16)
            nc.gpsimd.local_scatter(zf2[:], nz[:].bitcast(dt.int16),
                                    fidx2i[:], channels=P, num_elems=256,
                                    num_idxs=256)
            zsc = zf2[:].bitcast(dt.float32)

            # ---- fill coarse slots with uniform grid ----
            isCC = wt([P, M], "isCC")
            nc.gpsimd.tensor_scalar(isCC[:], zsc, 0.0, None, Alu.is_equal)
            cum2 = wt([P, M], "cum2")
            nc.vector.tensor_tensor_scan(cum2[:], isCC[:], zero_s[:],
                                         0.0, Alu.add, Alu.add)
            zcf = wt([P, M], "zcf")
            nc.vector.scalar_tensor_tensor(zcf[:], cum2[:], dz_c,
                                           bc(aoff_c, M), Alu.mult, Alu.add)
            Z = wt([P, M], "Zm")
            nc.gpsimd.tensor_tensor(Z[:], isCC[:], zcf[:], Alu.mult)
            nc.gpsimd.tensor_tensor(Z[:], Z[:], zsc, Alu.add)
            deltm = wt([P, M], "deltm")
            nc.gpsimd.tensor_tensor(deltm[:, 0:M - 1], Z[:, 1:M],
                                    Z[:, 0:M - 1], Alu.subtract)
            nc.gpsimd.tensor_copy(deltm[:, M - 1:M], sd_c)

            # ---- final MLP at merged z ----
            xyzm = wt([P, 3, M], "xyzm")
            for c in range(3):
                nc.vector.scalar_tensor_tensor(
                    xyzm[:, c, :], Z[:], rd_s[:, t, c:c + 1],
                    bc(ro_s[:, t, c:c + 1], M), Alu.mult, Alu.add)
            nc.vector.scalar_tensor_tensor(
                xyzm[:].rearrange("p c s -> p (c s)"),
                xyzm[:].rearrange("p c s -> p (c s)"),
                1.0, bc(neg1_c, 3 * M), Alu.min, Alu.max)
            state[t] = (xyzm, deltm)

        def phase2m(t):
            xyzm, deltm = state.pop(t)
            rhs7f = rhs7
            for c in range(3):
                nc.scalar.dma_start(f32r(rhs7f[2 * c:2 * c + 2, :]),
                                    f32r(xyzm[:, c, :]))

            # direction rows for the dterm accumulation matmul
            dT8_sb = wt([8, 64], "dT8sb")
            nc.scalar.dma_start(f32r(dT8_sb[:]), f32r(dT8_in[t]))

            rh1f = bpool.tile([128, fh], dt.float32, tag="rh1", name="rh1")
            for g in range(8):
                pA = ppool.tile([128, 1024], dt.float32, tag="mm", name="pmm")
                for ch in range(2):
                    nc.tensor.matmul(pA[:, 512 * ch:512 * (ch + 1)],
                                     f32r(lhsT7_s[:]),
                                     f32r(rhs7f[:, g * 1024 + 512 * ch:
                                                g * 1024 + 512 * (ch + 1)]),
                                     start=True, stop=True)
                nc.scalar.activation(f32r(rh1f[:, g * 1024:(g + 1) * 1024]),
                                     pA[:], Act.Relu)

            h20m = wt([P, M], "h20m")
            sgfi = gpool.tile([8, 2048], dt.float32, tag="sgfi", name="sgfi")
            for g in range(4):
                pS = pspool.tile([8, 512], dt.float32, tag="ps", name="pps")
                for ch in range(4):
                    nc.tensor.matmul(pS[:],
                                     f32r(w0p_s[:, ch, :]),
                                     f32r(rh1f[:, (4 * ch + g) * 512:
                                               (4 * ch + g + 1) * 512]),
                                     start=(ch == 0), stop=(ch == 3))
                nc.vector.tensor_copy(sgfi[:, 512 * g:512 * (g + 1)], pS[:])
            for h in range(2):
                nc.sync.dma_start(h20m[64 * h:64 * h + 64, :],
                                  sgfi[4 * h:4 * h + 4, :])

            # color hidden: relu(gc_pairs + dterm); dterm folded into the
            # PSUM accumulation via a second matmul with a broadcast rhs
            ch1 = bpool.tile([128, fh], dt.float32, tag="ch1", name="ch1")
            for g in range(8):
                pG = ppool.tile([128, 1024], dt.float32, tag="mm", name="pmm")
                for ch in range(2):
                    nc.tensor.matmul(pG[:, 512 * ch:512 * (ch + 1)],
                                     f32r(wgc_s[:]),
                                     f32r(rh1f[:, g * 1024 + 512 * ch:
                                               g * 1024 + 512 * (ch + 1)]),
                                     start=True, stop=False)
                    rs = 8 * g + 4 * ch
                    nc.tensor.matmul(
                        pG[:, 512 * ch:512 * (ch + 1)]
                        .rearrange("h (r s) -> h r s", s=M),
                        f32r(dlhs8_s[:]),
                        f32r(dT8_sb[:, rs:rs + 4]
                             .rearrange("p (r o) -> p r o", o=1)
                             .broadcast_to((8, 4, M))),
                        start=False, stop=True)
                nc.scalar.activation(f32r(ch1[:, g * 1024:(g + 1) * 1024]),
                                     pG[:], Act.Relu)

            # rgb pre-activation; SBUF rows ordered (h c k), DRAM bridge
            rgbs = dpool.tile([2, 3, 64, M], dt.float32, tag="rgbscr",
                              name="rgbscr")
            for g in range(4):
                pC = pspool.tile([24, 512], dt.float32, tag="ps", name="pps")
                for ch in range(4):
                    nc.tensor.matmul(pC[:],
                                     f32r(wc2_s[:, ch, :]),
                                     f32r(ch1[:, (4 * ch + g) * 512:
                                              (4 * ch + g + 1) * 512]),
                                     start=(ch == 0), stop=(ch == 3))
                rgb_sb = wt([24, 512], "rgbsb")
                nc.vector.tensor_copy(rgb_sb[:], pC[:])
                nc.sync.dma_start(
                    rgbs[:].rearrange("h c (k gg pl) s -> gg h c k (pl s)",
                                      gg=4, pl=4)[g],
                    rgb_sb[:])

            state[(t, 'f')] = (h20m, deltm, rgbs)

        def phase2c(t):
            h20m, deltm, rgbs = state.pop((t, 'f'))
            # ---- composite in rays layout ----
            sigm = wt([P, M], "sigm")
            nc.scalar.activation(sigm[:], h20m[:], Act.Exp, bias=bd2_0c)
            dsg2 = wt([P, M], "dsg2")
            nc.gpsimd.tensor_tensor(dsg2[:], deltm[:], sigm[:], Alu.mult)
            em2 = wt([P, M], "em2")
            nc.scalar.activation(em2[:], dsg2[:], Act.Exp, scale=-DS)
            sb2 = wt([P, M], "sb2")
            nc.gpsimd.memset(sb2[:, 0:1], 1.0)
            nc.scalar.activation(sb2[:, 1:M], em2[:, 0:M - 1],
                                 Act.Identity, bias=e15_c)
            Tm = wt([P, M], "Tm")
            nc.vector.tensor_tensor_scan(Tm[:], sb2[:], zero_s[:], 1.0,
                                         Alu.mult, Alu.add)
            alpm = wt([P, M], "alpm")
            nc.gpsimd.tensor_scalar(alpm[:], em2[:], -1.0, 1.0,
                                    Alu.mult, Alu.add)
            wm = wt([P, M], "wm")
            nc.gpsimd.tensor_tensor(wm[:], alpm[:], Tm[:], Alu.mult)
            wsum = wt([P, 1], "wsum")
            nc.vector.tensor_reduce(wsum[:], wm[:], AxX, Alu.add)
            wmm = wt([P, M], "wmm")
            nc.vector.scalar_tensor_tensor(wmm[:], wm[:], 1e-4, wm[:],
                                           Alu.is_gt, Alu.mult)

            rgbp = wt([P, 3, M], "rgbp")
            for h_ in range(2):
                nc.sync.dma_start(rgbp[64 * h_:64 * (h_ + 1), :, :],
                                  rgbs[h_].rearrange("c p s -> p c s"))
            nc.vector.tensor_tensor(
                rgbp[:], rgbp[:],
                bc2_s[:].rearrange("p (c o) -> p c o", o=1).broadcast_to((P, 3, M)),
                Alu.add)
            nc.scalar.activation(rgbp[:].rearrange("p c s -> p (c s)"),
                                 rgbp[:].rearrange("p c s -> p (c s)"),
                                 Act.Exp, scale=-1.0)
            nc.scalar.activation(rgbp[:].rearrange("p c s -> p (c s)"),
                                 rgbp[:].rearrange("p c s -> p (c s)"),
                                 Act.Identity, bias=ones_c)
            nc.vector.reciprocal(rgbp[:], rgbp[:])
            nc.vector.tensor_tensor(
                rgbp[:], rgbp[:],
                wmm[:].rearrange("p (o s) -> p o s", o=1).broadcast_to((P, 3, M)),
                Alu.mult)
            img = wt([P, 3], "img")
            nc.vector.tensor_reduce(img[:], rgbp[:], AxX, Alu.add)
            bgw = wt([P, 1], "bgw")
            nc.vector.tensor_scalar(bgw[:], wsum[:], -1.0, 1.0, Alu.mult,
                                    Alu.add)
            nc.vector.scalar_tensor_tensor(img[:], bg_s[:], bgw[:], img[:],
                                           Alu.mult, Alu.add)
            nc.sync.dma_start(img_out[:, t, :], img[:])

        def phase1(t):
            phase1a(t)
            phase1b(t)

        phase1a(0)
        phase1a(1)
        with tc.high_priority():
            phase1b(0)
        for t in range(T):
            if t >= 1 and t + 1 < T:
                phase1a(t + 1)
            phase2m(t)
            if t + 1 < T:
                with tc.high_priority(offset=300):
                    phase1b(t + 1)
            if t >= 1:
                phase2c(t - 1)
        phase2c(T - 1)

    nc.compile()
    return nc


def _host_constants(inputs):
    Wd1 = np.asarray(inputs["Wd1"], np.float32)
    bd1 = np.asarray(inputs["bd1"], np.float32)
    Wd2 = np.asarray(inputs["Wd2"], np.float32)
    bd2 = np.asarray(inputs["bd2"], np.float32)
    Wc1 = np.asarray(inputs["Wc1"], np.float32)
    bc1 = np.asarray(inputs["bc1"], np.float32)
    Wc2 = np.asarray(inputs["Wc2"], np.float32)
    bc2 = np.asarray(inputs["bc2"], np.float32)
    tval = float(np.asarray(inputs["time"]).reshape(()))

    W1 = Wd1[:3]
    b1p = bd1 + tval * Wd1[3]
    w0 = Wd2[:, 0:1]
    Wgc = (Wd2[:, 1:].astype(np.float64) @ Wc1[3:].astype(np.float64)) \
        .astype(np.float32)
    bgc = (bd2[1:].astype(np.float64) @ Wc1[3:].astype(np.float64)) \
        .astype(np.float32)
    bd2_0 = float(bd2[0])

    # row layout (xA xB yA yB zA zB 1) to match the direct xyz DMA
    lhsT7 = np.zeros((7, 128), np.float32)
    for c in range(3):
        lhsT7[2 * c, 0:64] = W1[c]
        lhsT7[2 * c + 1, 64:128] = W1[c]
    lhsT7[6, 0:64] = b1p
    lhsT7[6, 64:128] = b1p

    # sigma SBUF rows ordered (h k): row = 4*h + k
    w0oct = np.zeros((128, 4, 8), np.float32)
    for ch in range(4):
        w0oct[0:64, ch, ch:ch + 1] = w0
        w0oct[64:128, ch, 4 + ch:5 + ch] = w0

    wgcpair = np.zeros((128, 128), np.float32)
    wgcpair[0:64, 0:64] = Wgc
    wgcpair[64:128, 64:128] = Wgc

    # rgb SBUF rows ordered (h c k): row = 12*h + 4*c + k
    wc2oct = np.zeros((128, 4, 24), np.float32)
    for ch in range(4):
        for c in range(3):
            wc2oct[0:64, ch, 4 * c + ch] = Wc2[:, c]
            wc2oct[64:128, ch, 12 + 4 * c + ch] = Wc2[:, c]

    dlhs8 = np.zeros((8, 128), np.float32)
    dlhs8[0:3, 0:64] = Wc1[:3]
    dlhs8[3, 0:64] = bc1 + bgc
    dlhs8[4:7, 64:128] = Wc1[:3]
    dlhs8[7, 64:128] = bc1 + bgc

    v = np.linspace(0.0, 1.0, S, dtype=np.float32)
    return {
        "v128": np.broadcast_to(v, (P, S)).copy(),
        "iota_r": np.broadcast_to(np.arange(M, dtype=np.float32), (P, M)).copy(),
        "iotap1_r": np.broadcast_to(np.arange(1, M + 1, dtype=np.float32),
                                    (P, M)).copy(),
        "iotaev62": np.broadcast_to(np.arange(62, dtype=np.float32) * 2,
                                    (P, 62)).copy(),
        "zero128": np.zeros((P, M), np.float32),
        "cc": np.broadcast_to(
            np.array([1.0, -1.0, 1e-9, 1e-15, 1e-5, -1.0 / 128.0,
                      -16777216.0, -2.0, 16777216.0, 0, 0, 0], np.float32),
            (P, 12)).copy(),
        "ones_row": np.ones((1, P * M), np.float32),
        "lhsT7": lhsT7, "w0oct": w0oct, "wgcpair": wgcpair,
        "wc2oct": wc2oct, "dlhs8": dlhs8,
        "bgrep": np.broadcast_to(
            np.asarray(inputs["background_color"], np.float32), (P, 3)).copy(),
        "bc2rep": np.broadcast_to(bc2, (P, 3)).copy(),
        "scl": np.broadcast_to(
            np.array([bd2_0, 0, 0, 0], np.float32), (P, 4)).copy(),
    }


def kernel(**inputs):
    global _BUILT
    assert int(inputs["num_steps"]) == S
    assert int(inputs["upsample_steps"]) == U

    if _BUILT is None:
        _BUILT = _build()
    nc = _BUILT

    consts = _host_constants(inputs)
    ro = np.asarray(inputs["rays_o"], np.float32).reshape(NRAYS, 3)
    rd = np.asarray(inputs["rays_d"], np.float32).reshape(NRAYS, 3)

    in_maps = []
    for c in range(NCORES):
        sl_o = ro[c * R:(c + 1) * R].reshape(T, P, 3)
        sl_d = rd[c * R:(c + 1) * R].reshape(T, P, 3)
        dT8 = np.ones((T, 8, 64), np.float32)
        dT8[:, 0:3, :] = sl_d[:, 0:64, :].transpose(0, 2, 1)
        dT8[:, 4:7, :] = sl_d[:, 64:128, :].transpose(0, 2, 1)
        m = {
            "rays_o_k": np.ascontiguousarray(sl_o.transpose(1, 0, 2)),
            "rays_d_k": np.ascontiguousarray(sl_d.transpose(1, 0, 2)),
            "dT8_k": dT8,
        }
        m.update(consts)
        in_maps.append(m)

    res = run_bass_kernel_spmd(nc, in_maps, core_ids=list(range(NCORES)))
    global LAST_RESULT
    LAST_RESULT = res
    outs = []
    for c in range(NCORES):
        img = res.results[c]["img_k"]
        outs.append(img.transpose(1, 0, 2).reshape(R, 3))
    return np.concatenate(outs, 0).reshape(1, NRAYS, 3)

